# revision 73
# baseline (speedup 1.0000x reference)
"""Trainium2 Bass kernel for causal GQA attention with RoPE (dense_transformer).

Problem shapes (hardcoded): x [4, 2048, 2048] f32, Wq [2048,2048], Wk/Wv [2048,1024],
Wo [2048,2048], cos/sin [2048,128]. Output [4, 2048, 2048] f32.

Sharding: 8 cores = 4 batches x 2 kv-head groups. Core c handles batch b=c//2 and
head group g=c%2: kv heads [4g, 4g+4), q heads [8g, 8g+8), and Wo rows
[1024g, 1024g+1024). Each core projects K/V/Q for only its own heads over the
full sequence (no duplicated projection work), runs attention for its 8 q-heads
over all 2048 query tokens (causally balanced by construction), and computes a
partial o_proj with its half of Wo's rows. The two cores of a batch produce
additive partial outputs which the host sums - no device collectives.

The per-core program is identical across cores (SPMD); all per-core variation
is carried in the input data (weight slices, batch activations). 1/sqrt(HD) is
folded into Wq host-side so only one cos/sin table pair is shipped.

Matmuls run in bf16 (1 cyc/row on the PE vs 4 for fp32) with fp32 PSUM
accumulation. Layouts keep the contraction dim on partitions everywhere:
activations/projections live transposed ([feature, token]); x is streamed once
per core, producing K, V and Q per 512-token chunk; scores are computed per
q-block as S^T[key, q] so exp writes P^T directly; a ones-column appended to V
makes the AV matmul accumulate the softmax denominator for free; P^T feeds the
AV matmul whose [q, d] output is normalized, PE-transposed, and kept in SBUF
as AttnOut^T - the stationary operand of the final o_proj.

Scheduling: the attention work is one continuous stream of
(slot, kv-head, kv-block-pair) units, software-pipelined so the PE never waits
on the Activation engine (AV of unit u issues during unit u+1's scores, DVE
normalize one unit later, PE transpose one more unit later). Units of slots
0-7 are sprinkled between projection PSUM groups of chunks 1-2, hiding their
exp latency under pure-PE projection matmuls; their transposes (which need the
attnT buffer that only fits in SBUF after the projection pools release) are
deferred to the attention phase. o_proj quarters - pure PE work - are paced
one per ~3 stream units once Wo lands, keeping the PE fed through the
otherwise Act-bound tail slots. One shared [128, 512] f32 PSUM rotation serves
projection, scores, and o_proj accumulators, so phase transitions need no PSUM
pool drains.
"""

import sys

sys.path.insert(0, "/opt/trn_rl_repo")

import functools
import math
from contextlib import ExitStack

import ml_dtypes
import numpy as np

B, S, H = 4, 2048, 2048
NH, NKV, HD = 16, 8, 128
NKVg = NKV // 2        # kv heads per core: 4
NHg = NH // 2          # q heads per core: 8
QD = NHg * HD          # 1024
KVD = NKVg * HD        # 512
NSLOT = S // 128       # 16 q-blocks per core
NCHUNK = S // 512      # 4 token chunks for projections
NCORES = 8
NEG = -1.0e30
BF16 = ml_dtypes.bfloat16


def _build_program():
    import concourse.mybir as mybir
    import concourse.tile as tile
    from concourse import bacc
    from concourse.masks import make_identity

    dt = mybir.dt
    f32, bf16 = dt.float32, dt.bfloat16
    ADD, MULT = mybir.AluOpType.add, mybir.AluOpType.mult
    EXP = mybir.ActivationFunctionType.Exp
    nc = bacc.Bacc("TRN2", target_bir_lowering=False, debug=False)

    xT = nc.dram_tensor("xT", [H, S], bf16, kind="ExternalInput").ap()
    wq = nc.dram_tensor("wq", [H, QD], bf16, kind="ExternalInput").ap()
    wk = nc.dram_tensor("wk", [H, KVD], bf16, kind="ExternalInput").ap()
    wv = nc.dram_tensor("wv", [H, KVD], bf16, kind="ExternalInput").ap()
    wo = nc.dram_tensor("wo", [QD, H], bf16, kind="ExternalInput").ap()
    cosk = nc.dram_tensor("cosk", [HD, S], bf16, kind="ExternalInput").ap()
    sink = nc.dram_tensor("sink", [HD, S], bf16, kind="ExternalInput").ap()
    maskT = nc.dram_tensor("maskT", [128, 256], f32, kind="ExternalInput").ap()
    out = nc.dram_tensor("out", [S, H], bf16, kind="ExternalOutput").ap()

    xT_r = xT.rearrange("(a p) t -> p a t", p=128)     # [128, 16, S]
    wq_r = wq.rearrange("(a p) n -> p a n", p=128)     # [128, 16, QD]
    wk_r = wk.rearrange("(a p) n -> p a n", p=128)     # [128, 16, KVD]
    wv_r = wv.rearrange("(a p) n -> p a n", p=128)
    wo_r = wo.rearrange("(a p) n -> p a n", p=128)     # [128, 8, H]

    def rope(pool, ps, cos_sb, sin_sb, dst):
        # dst = ps*cos + rot64(ps)*sin  (sign of the rotation folded into sin).
        # The rotated reads keep ps in PSUM: only PSUM operands may sit at a
        # different start partition than the other operands.
        t1 = pool.tile([128, 512], bf16, tag="rope_t1")
        nc.vector.tensor_tensor(t1, ps, cos_sb, MULT)
        t2 = pool.tile([128, 512], bf16, tag="rope_t2")
        nc.vector.tensor_tensor(t2[0:64, :], ps[64:128, :], sin_sb[0:64, :], MULT)
        nc.vector.tensor_tensor(t2[64:128, :], ps[0:64, :], sin_sb[64:128, :], MULT)
        nc.gpsimd.tensor_tensor(dst, t1, t2, ADD)

    with tile.TileContext(nc) as tc, ExitStack() as top:
        misc = top.enter_context(tc.tile_pool(name="misc", bufs=1))
        ident = misc.tile([128, 128], bf16)
        make_identity(nc, ident)
        mask_sb = misc.tile([128, 256], f32)  # diagonal-block causal mask, x2 heads

        # Shared top-level PSUM pools: one [128, 512] f32 rotation serves the
        # projection, scores and o_proj accumulators (same bank footprint), so
        # phase transitions need no PSUM pool drains.
        ps512 = top.enter_context(tc.tile_pool(name="ps512", bufs=5, space="PSUM"))
        psot = top.enter_context(tc.tile_pool(name="ps_ot", bufs=2, space="PSUM"))
        psoT = top.enter_context(tc.tile_pool(name="ps_oT", bufs=1, space="PSUM"))

        kvq = top.enter_context(tc.tile_pool(name="kvq", bufs=1))
        kT_sb = kvq.tile([128, NKVg, S], bf16)    # K^T rope'd: [d, kvh, t]
        # V with a ones-column appended per kv head: [tok_p, tok_blk, kvh*129+d];
        # column 128 of each head accumulates the softmax denominator during AV.
        v_sb = kvq.tile([128, NSLOT, NKVg * (HD + 1)], bf16)
        # Q^T rope'd (1/sqrt(HD) folded into wq), [d, slot, head, qi] so a GQA
        # pair is one contiguous 256-wide moving operand per slot.
        qT_sb = kvq.tile([128, NSLOT, NHg, 128], bf16)
        for kvh in range(NKVg):
            nc.vector.memset(v_sb[:, :, kvh * 129 + 128:kvh * 129 + 129], 1.0)

        # Attention-stream pools and state live at top level: units of the
        # stream are SPRINKLED between projection PSUM groups of chunks 1-2
        # (slots 0-3 during chunk 1, slots 4-7 during chunk 2), hiding their
        # Activation-engine exp latency under pure-PE projection matmuls.
        # Transposes into attnT (and o_proj) wait until the attention phase,
        # when the projection-phase SBUF pools have been released; normalized
        # AV outputs queue in the small onp pool meanwhile.
        ptp = top.enter_context(tc.tile_pool(name="pT", bufs=8))
        stat = top.enter_context(tc.tile_pool(name="stat", bufs=8))
        onp = top.enter_context(tc.tile_pool(name="o_norm", bufs=34))

        ots = {}     # (s, kvh) -> [128, 2, HD+1] f32 PSUM accumulator
        onorms = {}  # (s, kvh) -> [128, 2, 128] bf16 normalized AV out
        pend_av = []
        norm_q = []
        t_q = []
        oproj_q = []
        sprinkle_q = []
        rate = [0]
        late = {}    # attnT/wo/op set once the attention phase opens
        released = [0]  # slots [0, released[0]) have o_proj quarters queued
        attn_ctr = [0]

        def do_oproj(tb, ncol):
            ps = ps512.tile([128, 512], f32, tag="ps512", name=f"pso_{tb}_{ncol}")
            for kt in range(8):
                nc.tensor.matmul(
                    ps,
                    late["attnT"][:, kt, tb * 128:(tb + 1) * 128],
                    late["wo"][:, kt, ncol * 512:(ncol + 1) * 512],
                    start=(kt == 0),
                    stop=(kt == 7),
                )
            st = late["op"].tile([128, 512], bf16)
            nc.scalar.copy(st, ps)
            nc.sync.dma_start(
                out=out[tb * 128:(tb + 1) * 128, ncol * 512:(ncol + 1) * 512],
                in_=st,
            )

        def do_av(s_, kvh_, pts, kls, kb0):
            nkb_ = s_ + 1
            ot = ots[(s_, kvh_)]
            # One PSUM accumulation group covers BOTH heads' slices of the
            # packed ot tile (a group is bank-granular): start only on the
            # very first matmul - its pending-zero mark gives j=1's first
            # write overwrite semantics - and stop only on the very last.
            for kl in range(kls):
                kb = kb0 + kl
                for j in range(2):
                    nc.tensor.matmul(
                        ot[:, j, :],
                        pts[:, kl * 256 + j * 128:kl * 256 + (j + 1) * 128],
                        v_sb[:, kb, kvh_ * 129:kvh_ * 129 + 129],
                        start=(kb == 0 and j == 0),
                        stop=(kb == nkb_ - 1 and j == 1),
                    )

        def do_norm(key):
            ot = ots.pop(key)
            onorm = onp.tile([128, 2, 128], bf16)
            for j in range(2):
                rec = stat.tile([128, 1], f32, tag="rec")
                nc.vector.reciprocal(rec, ot[:, j, HD:HD + 1])
                nc.vector.tensor_scalar_mul(onorm[:, j, :], ot[:, j, 0:HD], rec)
            onorms[key] = onorm

        def release_slots(upto):
            while released[0] < upto:
                oproj_q.extend((released[0], ncol) for ncol in range(4))
                released[0] += 1

        def do_transpose(key):
            s_, kvh_ = key
            onorm = onorms.pop(key)
            qs_ = slice(s_ * 128, (s_ + 1) * 128)
            oT = psoT.tile([128, 2, 128], bf16)
            for j in range(2):
                nc.tensor.transpose(oT[:, j, :], onorm[:, j, :], ident)
                dst = late["attnT"][:, 2 * kvh_ + j, qs_]
                if j == 0:
                    nc.vector.tensor_copy(dst, oT[:, j, :])
                else:
                    nc.scalar.copy(dst, oT[:, j, :])
            if kvh_ == 1:
                # release the previous slot only once this slot's pipeline
                # is underway, so o_proj never waits on a fresh attnT copy
                release_slots(s_)

        def drain_one():
            if t_q and "attnT" in late:
                do_transpose(t_q.pop(0))
            if norm_q:
                key = norm_q.pop(0)
                do_norm(key)
                t_q.append(key)

        def emit_unit(s, kvh, p):
            nkb = s + 1
            npair = (nkb + 1) // 2
            h0 = 2 * kvh
            if p == 0:
                ots[(s, kvh)] = psot.tile(
                    [128, 2, HD + 1], f32, tag="ot", name=f"ot_{s}_{kvh}"
                )
            kls = 2 if 2 * p + 1 < nkb else 1
            sT = ps512.tile([128, 512], f32, tag="ps512", name=f"sT_{s}_{kvh}_{p}")
            for kl in range(kls):
                kb = 2 * p + kl
                nc.tensor.matmul(
                    sT[:, kl * 256:(kl + 1) * 256],
                    kT_sb[:, kvh, kb * 128:(kb + 1) * 128],
                    qT_sb[:, s, h0:h0 + 2, :],
                    start=True,
                    stop=True,
                )
            if p == npair - 1:
                # diagonal block (kb == s) is last in this pair
                dsl = slice((kls - 1) * 256, kls * 256)
                nc.vector.tensor_tensor(sT[:, dsl], sT[:, dsl], mask_sb, ADD)
            pts = ptp.tile([128, 512], bf16)
            nc.scalar.activation(pts[:, 0:kls * 256], sT[:, 0:kls * 256], EXP)
            pend_av.append((s, kvh, pts, kls, 2 * p))
            if len(pend_av) > 4:
                prev = pend_av.pop(0)
                do_av(*prev)
                if prev[4] + prev[3] == prev[0] + 1:
                    # that AV was its (s, kvh)'s last kv-block
                    norm_q.append((prev[0], prev[1]))
            drain_one()
            if "attnT" in late:
                # Pace o_proj quarters: none before wo lands (~24 units in),
                # then one per 3 units so the backlog covers the Act-bound
                # stretches of the long slots.
                attn_ctr[0] += 1
                if oproj_q and attn_ctr[0] >= 24 and attn_ctr[0] % 3 == 0:
                    do_oproj(*oproj_q.pop(0))

        def sprinkle():
            for _ in range(rate[0]):
                if sprinkle_q:
                    emit_unit(*sprinkle_q.pop(0))

        # ---- Phase 1: K/V/Q projections (+RoPE), single pass over x ----
        # DMA order minimizes the startup bubble: the first K matmuls need
        # only wk's first head slice + x chunk 0, so those ship first.
        with ExitStack() as ph:
            xp = ph.enter_context(tc.tile_pool(name="x_in", bufs=2))
            wp = ph.enter_context(tc.tile_pool(name="w_kvq", bufs=1))
            csp = ph.enter_context(tc.tile_pool(name="cs", bufs=1))
            rp = ph.enter_context(tc.tile_pool(name="rope_t", bufs=3))
            wkc = wp.tile([128, 16, KVD], bf16, tag="wk")
            nc.sync.dma_start(out=wkc[:, :, 0:128], in_=wk_r[:, :, 0:128])
            xc0 = xp.tile([128, 16, 512], bf16, tag="xc")
            nc.sync.dma_start(out=xc0[:, 0:4, :], in_=xT_r[:, 0:4, 0:512])
            nc.sync.dma_start(out=xc0[:, 4:8, :], in_=xT_r[:, 4:8, 0:512])
            nc.sync.dma_start(out=wkc[:, :, 128:KVD], in_=wk_r[:, :, 128:KVD])
            nc.sync.dma_start(out=xc0[:, 8:16, :], in_=xT_r[:, 8:16, 0:512])
            cos_sb = csp.tile([128, S], bf16, tag="cos")
            nc.sync.dma_start(out=cos_sb, in_=cosk)
            sin_sb = csp.tile([128, S], bf16, tag="sin")
            nc.sync.dma_start(out=sin_sb, in_=sink)
            # wq split around wv so chunk 0's Q (first half) and V never wait
            wq_lo = wp.tile([128, 16, QD // 2], bf16, tag="wq_lo")
            nc.sync.dma_start(out=wq_lo, in_=wq_r[:, :, 0:QD // 2])
            wvc = wp.tile([128, 16, KVD], bf16, tag="wv")
            nc.sync.dma_start(out=wvc, in_=wv_r)
            wq_hi = wp.tile([128, 16, QD // 2], bf16, tag="wq_hi")
            nc.sync.dma_start(out=wq_hi, in_=wq_r[:, :, QD // 2:QD])

            def proj_k(xc, cc):
                for kvh in range(NKVg):
                    ps = ps512.tile([128, 512], f32, tag="ps512", name=f"psk_{cc.start}_{kvh}")
                    for kt in range(16):
                        nc.tensor.matmul(
                            ps,
                            wkc[:, kt, kvh * 128:(kvh + 1) * 128],
                            xc[:, kt, :],
                            start=(kt == 0),
                            stop=(kt == 15),
                        )
                    rope(rp, ps, cos_sb[:, cc], sin_sb[:, cc], kT_sb[:, kvh, cc])
                    sprinkle()

            def proj_v(xc, c):
                for tbl in range(4):
                    tb = c * 4 + tbl
                    ps = ps512.tile([128, 512], f32, tag="ps512", name=f"psv_{tb}")
                    for kt in range(16):
                        nc.tensor.matmul(
                            ps,
                            xc[:, kt, tbl * 128:(tbl + 1) * 128],
                            wvc[:, kt, :],
                            start=(kt == 0),
                            stop=(kt == 15),
                        )
                    for kvh in range(NKVg):
                        dst = v_sb[:, tb, kvh * 129:kvh * 129 + 128]
                        src = ps[:, kvh * 128:(kvh + 1) * 128]
                        if kvh % 2 == 0:
                            nc.vector.tensor_copy(dst, src)
                        else:
                            nc.scalar.copy(dst, src)
                    sprinkle()

            def proj_q(xc, cc, c, hs):
                for h in hs:
                    wq_half = wq_lo if h < 4 else wq_hi
                    ps = ps512.tile([128, 512], f32, tag="ps512", name=f"psq_{c}_{h}")
                    for kt in range(16):
                        nc.tensor.matmul(
                            ps,
                            wq_half[:, kt, (h % 4) * 128:(h % 4 + 1) * 128],
                            xc[:, kt, :],
                            start=(kt == 0),
                            stop=(kt == 15),
                        )
                    rope(
                        rp, ps, cos_sb[:, cc], sin_sb[:, cc],
                        qT_sb[:, 4 * c:4 * c + 4, h, :],
                    )
                    sprinkle()

            for c in range(NCHUNK):
                cc = slice(c * 512, (c + 1) * 512)
                if c == 0:
                    xc = xc0
                else:
                    xc = xp.tile([128, 16, 512], bf16, tag="xc")
                    nc.sync.dma_start(out=xc, in_=xT_r[:, :, cc])
                proj_k(xc, cc)
                if c == 0:
                    # chunk 0 ordered around DMA arrival: K, Q-lo, V, Q-hi
                    proj_q(xc, cc, c, range(4))
                    proj_v(xc, c)
                    proj_q(xc, cc, c, range(4, 8))
                    # mask ships here: off the startup critical path, ahead
                    # of the first sprinkled diagonal unit in chunk 1
                    nc.sync.dma_start(out=mask_sb, in_=maskT)
                else:
                    proj_v(xc, c)
                    proj_q(xc, cc, c, range(8))
                if c < 2:
                    # queue the attention units of the slots this chunk
                    # completed; they sprinkle into the NEXT chunk's groups
                    for s in range(4 * c, 4 * c + 4):
                        npair = (s + 2) // 2
                        sprinkle_q.extend(
                            (s, kvh, p)
                            for kvh in range(NKVg)
                            for p in range(npair)
                        )
                    rate[0] = 2 if c == 0 else 4

        # ---- Phase 2: attention stream for slots 8-15 + o_proj ----
        attn = top.enter_context(tc.tile_pool(name="attn_res", bufs=1))
        attnT_sb = attn.tile([128, NHg, S], bf16)     # [d, head, t]
        wo_sb = attn.tile([128, 8, H], bf16)
        nc.sync.dma_start(out=wo_sb, in_=wo_r)
        with ExitStack() as ph:
            op = ph.enter_context(tc.tile_pool(name="o_stage", bufs=6))
            late["attnT"] = attnT_sb
            late["wo"] = wo_sb
            late["op"] = op
            while sprinkle_q:  # leftovers if chunk pacing undershot
                emit_unit(*sprinkle_q.pop(0))
            for s in range(8, NSLOT):
                npair = (s + 2) // 2
                for kvh in range(NKVg):
                    for p in range(npair):
                        emit_unit(s, kvh, p)
            # flush the pipeline
            while pend_av:
                prev = pend_av.pop(0)
                do_av(*prev)
                if prev[4] + prev[3] == prev[0] + 1:
                    norm_q.append((prev[0], prev[1]))
            while norm_q or t_q or oproj_q or released[0] < NSLOT:
                drain_one()
                if released[0] == NSLOT - 1 and not (norm_q or t_q):
                    release_slots(NSLOT)
                if oproj_q:
                    do_oproj(*oproj_q.pop(0))
                elif released[0] < NSLOT - 1:
                    release_slots(released[0] + 1)
    nc.compile()
    return nc


@functools.lru_cache(maxsize=1)
def _program():
    return _build_program()


def _host_prep(x, cos, sin, Wq, Wk, Wv, Wo):
    x = np.asarray(x, dtype=np.float32)
    cos = np.asarray(cos, dtype=np.float32)
    sin = np.asarray(sin, dtype=np.float32)
    scale = 1.0 / math.sqrt(HD)

    cosT = np.ascontiguousarray(cos.T)            # [HD, S]
    sinT_eff = np.ascontiguousarray(sin.T)
    sinT_eff[: HD // 2] *= -1.0                   # fold rotate_half signs

    # diagonal-block causal mask: [key j, q i] allowed j <= i, tiled x2 heads
    ii = np.arange(128)[None, :]
    jj = np.arange(128)[:, None]
    m = np.where(jj <= ii, 0.0, NEG).astype(np.float32)   # [key, q]
    maskT = np.ascontiguousarray(np.concatenate([m, m], axis=1))  # [128, 256]

    wq_f = np.asarray(Wq, dtype=np.float32) * scale
    wqs = [np.ascontiguousarray(wq_f[:, g * QD:(g + 1) * QD]).astype(BF16)
           for g in range(2)]
    wk_f = np.asarray(Wk, dtype=np.float32)
    wks = [np.ascontiguousarray(wk_f[:, g * KVD:(g + 1) * KVD]).astype(BF16)
           for g in range(2)]
    wv_f = np.asarray(Wv, dtype=np.float32)
    wvs = [np.ascontiguousarray(wv_f[:, g * KVD:(g + 1) * KVD]).astype(BF16)
           for g in range(2)]
    wo_f = np.asarray(Wo, dtype=np.float32)
    wos = [np.ascontiguousarray(wo_f[g * QD:(g + 1) * QD, :]).astype(BF16)
           for g in range(2)]

    in_maps = []
    for c in range(NCORES):
        b, g = c // 2, c % 2
        xT_b = np.ascontiguousarray(x[b].T).astype(BF16)   # [H, S]
        in_maps.append(
            {
                "xT": xT_b,
                "wq": wqs[g],
                "wk": wks[g],
                "wv": wvs[g],
                "wo": wos[g],
                "cosk": cosT.astype(BF16),
                "sink": sinT_eff.astype(BF16),
                "maskT": maskT,
            }
        )
    return in_maps, None


def _assemble(results):
    full = np.empty((B, S, H), dtype=np.float32)
    for b in range(B):
        np.add(
            results[2 * b]["out"].astype(np.float32),
            results[2 * b + 1]["out"].astype(np.float32),
            out=full[b],
        )
    return full


LAST_RESULTS = None


def kernel(x, cos, sin, Wq, Wk, Wv, Wo, _trace=False):
    global LAST_RESULTS
    from concourse.bass_utils import run_bass_kernel_spmd

    in_maps, _ = _host_prep(x, cos, sin, Wq, Wk, Wv, Wo)
    res = run_bass_kernel_spmd(
        _program(),
        in_maps,
        core_ids=list(range(NCORES)),
        trace=_trace,
        trace_cores=list(range(NCORES)) if _trace else None,
    )
    LAST_RESULTS = res
    return _assemble(res.results)


# revision 75
# speedup vs baseline: 1.2966x; 1.2966x over previous
"""Trainium2 Bass kernel for causal GQA attention with RoPE (dense_transformer).

Problem shapes (hardcoded): x [4, 2048, 2048] f32, Wq [2048,2048], Wk/Wv [2048,1024],
Wo [2048,2048], cos/sin [2048,128]. Output [4, 2048, 2048] f32.

Sharding: 8 cores = 4 batches x 2 kv-head groups. Core c handles batch b=c//2 and
head group g=c%2: kv heads [4g, 4g+4), q heads [8g, 8g+8), and Wo rows
[1024g, 1024g+1024). Each core projects K/V/Q for only its own heads over the
full sequence (no duplicated projection work), runs attention for its 8 q-heads
over all 2048 query tokens (causally balanced by construction), and computes a
partial o_proj with its half of Wo's rows. The two cores of a batch produce
additive partial outputs which the host sums - no device collectives.

The per-core program is identical across cores (SPMD); all per-core variation
is carried in the input data (weight slices, batch activations). 1/sqrt(HD) is
folded into Wq host-side so only one cos/sin table pair is shipped.

Matmuls run in bf16 (1 cyc/row on the PE vs 4 for fp32) with fp32 PSUM
accumulation. Layouts keep the contraction dim on partitions everywhere:
activations/projections live transposed ([feature, token]); x is streamed once
per core, producing K, V and Q per 512-token chunk; scores are computed per
q-block as S^T[key, q] so exp writes P^T directly; a ones-column appended to V
makes the AV matmul accumulate the softmax denominator for free; P^T feeds the
AV matmul whose [q, d] output is normalized, PE-transposed, and kept in SBUF
as AttnOut^T - the stationary operand of the final o_proj.

Scheduling: the attention work is one continuous stream of
(slot, kv-head, kv-block-pair) units, software-pipelined so the PE never waits
on the Activation engine (AV of unit u issues during unit u+1's scores, DVE
normalize one unit later, PE transpose one more unit later). Units of slots
0-7 are sprinkled between projection PSUM groups of chunks 1-2, hiding their
exp latency under pure-PE projection matmuls; their transposes (which need the
attnT buffer that only fits in SBUF after the projection pools release) are
deferred to the attention phase. o_proj quarters - pure PE work - are paced
one per ~3 stream units once Wo lands, keeping the PE fed through the
otherwise Act-bound tail slots. One shared [128, 512] f32 PSUM rotation serves
projection, scores, and o_proj accumulators, so phase transitions need no PSUM
pool drains.
"""

import sys

sys.path.insert(0, "/opt/trn_rl_repo")

import functools
import math
from contextlib import ExitStack

import ml_dtypes
import numpy as np

B, S, H = 4, 2048, 2048
NH, NKV, HD = 16, 8, 128
NKVg = NKV // 2        # kv heads per core: 4
NHg = NH // 2          # q heads per core: 8
QD = NHg * HD          # 1024
KVD = NKVg * HD        # 512
NSLOT = S // 128       # 16 q-blocks per core
NCHUNK = S // 512      # 4 token chunks for projections
NCORES = 8
NEG = -1.0e30
BF16 = ml_dtypes.bfloat16


def _build_program():
    import concourse.mybir as mybir
    import concourse.tile as tile
    from concourse import bacc
    from concourse.masks import make_identity

    dt = mybir.dt
    f32, bf16 = dt.float32, dt.bfloat16
    ADD, MULT = mybir.AluOpType.add, mybir.AluOpType.mult
    EXP = mybir.ActivationFunctionType.Exp
    nc = bacc.Bacc("TRN2", target_bir_lowering=False, debug=False)

    xT = nc.dram_tensor("xT", [H, S], bf16, kind="ExternalInput").ap()
    wq = nc.dram_tensor("wq", [H, QD], bf16, kind="ExternalInput").ap()
    wk = nc.dram_tensor("wk", [H, KVD], bf16, kind="ExternalInput").ap()
    wv = nc.dram_tensor("wv", [H, KVD], bf16, kind="ExternalInput").ap()
    wo = nc.dram_tensor("wo", [QD, H], bf16, kind="ExternalInput").ap()
    cosk = nc.dram_tensor("cosk", [HD, S], bf16, kind="ExternalInput").ap()
    sink = nc.dram_tensor("sink", [HD, S], bf16, kind="ExternalInput").ap()
    maskT = nc.dram_tensor("maskT", [128, 256], f32, kind="ExternalInput").ap()
    out = nc.dram_tensor("out", [S, H], bf16, kind="ExternalOutput").ap()

    xT_r = xT.rearrange("(a p) t -> p a t", p=128)     # [128, 16, S]
    wq_r = wq.rearrange("(a p) n -> p a n", p=128)     # [128, 16, QD]
    wk_r = wk.rearrange("(a p) n -> p a n", p=128)     # [128, 16, KVD]
    wv_r = wv.rearrange("(a p) n -> p a n", p=128)
    wo_r = wo.rearrange("(a p) n -> p a n", p=128)     # [128, 8, H]

    def rope(pool, ps, cos_sb, sin_sb, dst):
        # dst = ps*cos + rot64(ps)*sin  (sign of the rotation folded into sin).
        # The rotated reads keep ps in PSUM: only PSUM operands may sit at a
        # different start partition than the other operands.
        t1 = pool.tile([128, 512], bf16, tag="rope_t1")
        nc.vector.tensor_tensor(t1, ps, cos_sb, MULT)
        t2 = pool.tile([128, 512], bf16, tag="rope_t2")
        nc.vector.tensor_tensor(t2[0:64, :], ps[64:128, :], sin_sb[0:64, :], MULT)
        nc.vector.tensor_tensor(t2[64:128, :], ps[0:64, :], sin_sb[64:128, :], MULT)
        nc.gpsimd.tensor_tensor(dst, t1, t2, ADD)

    with tile.TileContext(nc) as tc, ExitStack() as top:
        misc = top.enter_context(tc.tile_pool(name="misc", bufs=1))
        ident = misc.tile([128, 128], bf16)
        make_identity(nc, ident)
        mask_sb = misc.tile([128, 256], f32)  # diagonal-block causal mask, x2 heads

        # Shared top-level PSUM pools: one [128, 512] f32 rotation serves the
        # projection, scores and o_proj accumulators (same bank footprint), so
        # phase transitions need no PSUM pool drains.
        ps512 = top.enter_context(tc.tile_pool(name="ps512", bufs=5, space="PSUM"))
        psot = top.enter_context(tc.tile_pool(name="ps_ot", bufs=2, space="PSUM"))
        psoT = top.enter_context(tc.tile_pool(name="ps_oT", bufs=1, space="PSUM"))

        kvq = top.enter_context(tc.tile_pool(name="kvq", bufs=1))
        kT_sb = kvq.tile([128, NKVg, S], bf16)    # K^T rope'd: [d, kvh, t]
        # V with a ones-column appended per kv head: [tok_p, tok_blk, kvh*129+d];
        # column 128 of each head accumulates the softmax denominator during AV.
        v_sb = kvq.tile([128, NSLOT, NKVg * (HD + 1)], bf16)
        # Q^T rope'd (1/sqrt(HD) folded into wq), [d, slot, head, qi] so a GQA
        # pair is one contiguous 256-wide moving operand per slot.
        qT_sb = kvq.tile([128, NSLOT, NHg, 128], bf16)
        for kvh in range(NKVg):
            nc.vector.memset(v_sb[:, :, kvh * 129 + 128:kvh * 129 + 129], 1.0)

        # Attention-stream pools and state live at top level: units of the
        # stream are SPRINKLED between projection PSUM groups of chunks 1-2
        # (slots 0-3 during chunk 1, slots 4-7 during chunk 2), hiding their
        # Activation-engine exp latency under pure-PE projection matmuls.
        # Transposes into attnT (and o_proj) wait until the attention phase,
        # when the projection-phase SBUF pools have been released; normalized
        # AV outputs queue in the small onp pool meanwhile.
        ptp = top.enter_context(tc.tile_pool(name="pT", bufs=8))
        stat = top.enter_context(tc.tile_pool(name="stat", bufs=8))
        onp = top.enter_context(tc.tile_pool(name="o_norm", bufs=34))

        ots = {}     # (s, kvh) -> [128, 2, HD+1] f32 PSUM accumulator
        onorms = {}  # (s, kvh) -> [128, 2, 128] bf16 normalized AV out
        pend_av = []
        norm_q = []
        t_q = []
        oproj_q = []
        sprinkle_q = []
        rate = [0]
        late = {}    # attnT/wo/op set once the attention phase opens
        released = [0]  # slots [0, released[0]) have o_proj quarters queued
        attn_ctr = [0]

        def do_oproj(tb, ncol):
            ps = ps512.tile([128, 512], f32, tag="ps512", name=f"pso_{tb}_{ncol}")
            for kt in range(8):
                nc.tensor.matmul(
                    ps,
                    late["attnT"][:, kt, tb * 128:(tb + 1) * 128],
                    late["wo"][:, kt, ncol * 512:(ncol + 1) * 512],
                    start=(kt == 0),
                    stop=(kt == 7),
                )
            st = late["op"].tile([128, 512], bf16)
            nc.scalar.copy(st, ps)
            nc.sync.dma_start(
                out=out[tb * 128:(tb + 1) * 128, ncol * 512:(ncol + 1) * 512],
                in_=st,
            )

        def do_av(s_, kvh_, pts, kls, kb0):
            nkb_ = s_ + 1
            ot = ots[(s_, kvh_)]
            # One PSUM accumulation group covers BOTH heads' slices of the
            # packed ot tile (a group is bank-granular): start only on the
            # very first matmul - its pending-zero mark gives j=1's first
            # write overwrite semantics - and stop only on the very last.
            for kl in range(kls):
                kb = kb0 + kl
                for j in range(2):
                    nc.tensor.matmul(
                        ot[:, j, :],
                        pts[:, kl * 256 + j * 128:kl * 256 + (j + 1) * 128],
                        v_sb[:, kb, kvh_ * 129:kvh_ * 129 + 129],
                        start=(kb == 0 and j == 0),
                        stop=(kb == nkb_ - 1 and j == 1),
                    )

        def do_norm(key):
            ot = ots.pop(key)
            onorm = onp.tile([128, 2, 128], bf16)
            for j in range(2):
                rec = stat.tile([128, 1], f32, tag="rec")
                nc.vector.reciprocal(rec, ot[:, j, HD:HD + 1])
                nc.vector.tensor_scalar_mul(onorm[:, j, :], ot[:, j, 0:HD], rec)
            onorms[key] = onorm

        def release_slots(upto):
            while released[0] < upto:
                oproj_q.extend((released[0], ncol) for ncol in range(4))
                released[0] += 1

        def do_transpose(key):
            s_, kvh_ = key
            onorm = onorms.pop(key)
            qs_ = slice(s_ * 128, (s_ + 1) * 128)
            oT = psoT.tile([128, 2, 128], bf16)
            for j in range(2):
                nc.tensor.transpose(oT[:, j, :], onorm[:, j, :], ident)
                dst = late["attnT"][:, 2 * kvh_ + j, qs_]
                if j == 0:
                    nc.vector.tensor_copy(dst, oT[:, j, :])
                else:
                    nc.scalar.copy(dst, oT[:, j, :])
            if kvh_ == 1:
                # release the previous slot only once this slot's pipeline
                # is underway, so o_proj never waits on a fresh attnT copy
                release_slots(s_)

        def drain_one():
            if t_q and "attnT" in late:
                do_transpose(t_q.pop(0))
            if norm_q:
                key = norm_q.pop(0)
                do_norm(key)
                t_q.append(key)

        def emit_unit(s, kvh, p):
            nkb = s + 1
            npair = (nkb + 1) // 2
            h0 = 2 * kvh
            if p == 0:
                ots[(s, kvh)] = psot.tile(
                    [128, 2, HD + 1], f32, tag="ot", name=f"ot_{s}_{kvh}"
                )
            kls = 2 if 2 * p + 1 < nkb else 1
            sT = ps512.tile([128, 512], f32, tag="ps512", name=f"sT_{s}_{kvh}_{p}")
            for kl in range(kls):
                kb = 2 * p + kl
                nc.tensor.matmul(
                    sT[:, kl * 256:(kl + 1) * 256],
                    kT_sb[:, kvh, kb * 128:(kb + 1) * 128],
                    qT_sb[:, s, h0:h0 + 2, :],
                    start=True,
                    stop=True,
                )
            if p == npair - 1:
                # diagonal block (kb == s) is last in this pair
                dsl = slice((kls - 1) * 256, kls * 256)
                nc.vector.tensor_tensor(sT[:, dsl], sT[:, dsl], mask_sb, ADD)
            pts = ptp.tile([128, 512], bf16)
            nc.scalar.activation(pts[:, 0:kls * 256], sT[:, 0:kls * 256], EXP)
            pend_av.append((s, kvh, pts, kls, 2 * p))
            if len(pend_av) > 4:
                prev = pend_av.pop(0)
                do_av(*prev)
                if prev[4] + prev[3] == prev[0] + 1:
                    # that AV was its (s, kvh)'s last kv-block
                    norm_q.append((prev[0], prev[1]))
            drain_one()
            if "attnT" in late:
                # Pace o_proj quarters: none before wo lands (~24 units in),
                # then one per 3 units so the backlog covers the Act-bound
                # stretches of the long slots.
                attn_ctr[0] += 1
                if oproj_q and attn_ctr[0] >= 24 and attn_ctr[0] % 3 == 0:
                    do_oproj(*oproj_q.pop(0))

        def sprinkle():
            for _ in range(rate[0]):
                if sprinkle_q:
                    emit_unit(*sprinkle_q.pop(0))

        # ---- Phase 1: K/V/Q projections (+RoPE), single pass over x ----
        # DMA order minimizes the startup bubble: the first K matmuls need
        # only wk's first head slice + x chunk 0, so those ship first.
        with ExitStack() as ph:
            xp = ph.enter_context(tc.tile_pool(name="x_in", bufs=2))
            wp = ph.enter_context(tc.tile_pool(name="w_kvq", bufs=1))
            csp = ph.enter_context(tc.tile_pool(name="cs", bufs=1))
            rp = ph.enter_context(tc.tile_pool(name="rope_t", bufs=3))
            wkc = wp.tile([128, 16, KVD], bf16, tag="wk")
            nc.sync.dma_start(out=wkc[:, :, 0:128], in_=wk_r[:, :, 0:128])
            xc0 = xp.tile([128, 16, 512], bf16, tag="xc")
            nc.sync.dma_start(out=xc0[:, 0:4, :], in_=xT_r[:, 0:4, 0:512])
            nc.sync.dma_start(out=xc0[:, 4:8, :], in_=xT_r[:, 4:8, 0:512])
            nc.sync.dma_start(out=wkc[:, :, 128:KVD], in_=wk_r[:, :, 128:KVD])
            nc.sync.dma_start(out=xc0[:, 8:16, :], in_=xT_r[:, 8:16, 0:512])
            cos_sb = csp.tile([128, S], bf16, tag="cos")
            nc.sync.dma_start(out=cos_sb, in_=cosk)
            sin_sb = csp.tile([128, S], bf16, tag="sin")
            nc.sync.dma_start(out=sin_sb, in_=sink)
            # wq split around wv so chunk 0's Q (first half) and V never wait
            wq_lo = wp.tile([128, 16, QD // 2], bf16, tag="wq_lo")
            nc.sync.dma_start(out=wq_lo, in_=wq_r[:, :, 0:QD // 2])
            wvc = wp.tile([128, 16, KVD], bf16, tag="wv")
            nc.sync.dma_start(out=wvc, in_=wv_r)
            wq_hi = wp.tile([128, 16, QD // 2], bf16, tag="wq_hi")
            nc.sync.dma_start(out=wq_hi, in_=wq_r[:, :, QD // 2:QD])

            def proj_k(xc, cc):
                for kvh in range(NKVg):
                    ps = ps512.tile([128, 512], f32, tag="ps512", name=f"psk_{cc.start}_{kvh}")
                    for kt in range(16):
                        nc.tensor.matmul(
                            ps,
                            wkc[:, kt, kvh * 128:(kvh + 1) * 128],
                            xc[:, kt, :],
                            start=(kt == 0),
                            stop=(kt == 15),
                        )
                    rope(rp, ps, cos_sb[:, cc], sin_sb[:, cc], kT_sb[:, kvh, cc])
                    sprinkle()

            def proj_v(xc, c):
                for tbl in range(4):
                    tb = c * 4 + tbl
                    ps = ps512.tile([128, 512], f32, tag="ps512", name=f"psv_{tb}")
                    for kt in range(16):
                        nc.tensor.matmul(
                            ps,
                            xc[:, kt, tbl * 128:(tbl + 1) * 128],
                            wvc[:, kt, :],
                            start=(kt == 0),
                            stop=(kt == 15),
                        )
                    for kvh in range(NKVg):
                        dst = v_sb[:, tb, kvh * 129:kvh * 129 + 128]
                        src = ps[:, kvh * 128:(kvh + 1) * 128]
                        if kvh % 2 == 0:
                            nc.vector.tensor_copy(dst, src)
                        else:
                            nc.scalar.copy(dst, src)
                    sprinkle()

            def proj_q(xc, cc, c, hs):
                for h in hs:
                    wq_half = wq_lo if h < 4 else wq_hi
                    ps = ps512.tile([128, 512], f32, tag="ps512", name=f"psq_{c}_{h}")
                    for kt in range(16):
                        nc.tensor.matmul(
                            ps,
                            wq_half[:, kt, (h % 4) * 128:(h % 4 + 1) * 128],
                            xc[:, kt, :],
                            start=(kt == 0),
                            stop=(kt == 15),
                        )
                    rope(
                        rp, ps, cos_sb[:, cc], sin_sb[:, cc],
                        qT_sb[:, 4 * c:4 * c + 4, h, :],
                    )
                    sprinkle()

            for c in range(NCHUNK):
                cc = slice(c * 512, (c + 1) * 512)
                if c == 0:
                    xc = xc0
                else:
                    xc = xp.tile([128, 16, 512], bf16, tag="xc")
                    nc.sync.dma_start(out=xc, in_=xT_r[:, :, cc])
                proj_k(xc, cc)
                if c == 0:
                    # chunk 0 ordered around DMA arrival: K, Q-lo, V, Q-hi
                    proj_q(xc, cc, c, range(4))
                    proj_v(xc, c)
                    proj_q(xc, cc, c, range(4, 8))
                    # mask ships here: off the startup critical path, ahead
                    # of the first sprinkled diagonal unit in chunk 1
                    nc.sync.dma_start(out=mask_sb, in_=maskT)
                else:
                    proj_v(xc, c)
                    proj_q(xc, cc, c, range(8))
                if c < 2:
                    # queue the attention units of the slots this chunk
                    # completed; they sprinkle into the NEXT chunk's groups
                    for s in range(4 * c, 4 * c + 4):
                        npair = (s + 2) // 2
                        sprinkle_q.extend(
                            (s, kvh, p)
                            for kvh in range(NKVg)
                            for p in range(npair)
                        )
                    rate[0] = 2 if c == 0 else 4

        # ---- Phase 2: attention stream for slots 8-15 + o_proj ----
        attn = top.enter_context(tc.tile_pool(name="attn_res", bufs=1))
        attnT_sb = attn.tile([128, NHg, S], bf16)     # [d, head, t]
        wo_sb = attn.tile([128, 8, H], bf16)
        nc.sync.dma_start(out=wo_sb, in_=wo_r)
        with ExitStack() as ph:
            op = ph.enter_context(tc.tile_pool(name="o_stage", bufs=6))
            late["attnT"] = attnT_sb
            late["wo"] = wo_sb
            late["op"] = op
            while sprinkle_q:  # leftovers if chunk pacing undershot
                emit_unit(*sprinkle_q.pop(0))
            for s in range(8, NSLOT):
                npair = (s + 2) // 2
                for kvh in range(NKVg):
                    for p in range(npair):
                        emit_unit(s, kvh, p)
            # flush the pipeline
            while pend_av:
                prev = pend_av.pop(0)
                do_av(*prev)
                if prev[4] + prev[3] == prev[0] + 1:
                    norm_q.append((prev[0], prev[1]))
            while norm_q or t_q or oproj_q or released[0] < NSLOT:
                drain_one()
                if released[0] == NSLOT - 1 and not (norm_q or t_q):
                    release_slots(NSLOT)
                if oproj_q:
                    do_oproj(*oproj_q.pop(0))
                elif released[0] < NSLOT - 1:
                    release_slots(released[0] + 1)
    nc.compile()
    return nc


@functools.lru_cache(maxsize=1)
def _program():
    return _build_program()


def _host_prep(x, cos, sin, Wq, Wk, Wv, Wo):
    x = np.asarray(x, dtype=np.float32)
    cos = np.asarray(cos, dtype=np.float32)
    sin = np.asarray(sin, dtype=np.float32)
    scale = 1.0 / math.sqrt(HD)

    cosT = np.ascontiguousarray(cos.T)            # [HD, S]
    sinT_eff = np.ascontiguousarray(sin.T)
    sinT_eff[: HD // 2] *= -1.0                   # fold rotate_half signs

    # diagonal-block causal mask: [key j, q i] allowed j <= i, tiled x2 heads
    ii = np.arange(128)[None, :]
    jj = np.arange(128)[:, None]
    m = np.where(jj <= ii, 0.0, NEG).astype(np.float32)   # [key, q]
    maskT = np.ascontiguousarray(np.concatenate([m, m], axis=1))  # [128, 256]

    wq_f = np.asarray(Wq, dtype=np.float32) * scale
    wqs = [np.ascontiguousarray(wq_f[:, g * QD:(g + 1) * QD]).astype(BF16)
           for g in range(2)]
    wk_f = np.asarray(Wk, dtype=np.float32)
    wks = [np.ascontiguousarray(wk_f[:, g * KVD:(g + 1) * KVD]).astype(BF16)
           for g in range(2)]
    wv_f = np.asarray(Wv, dtype=np.float32)
    wvs = [np.ascontiguousarray(wv_f[:, g * KVD:(g + 1) * KVD]).astype(BF16)
           for g in range(2)]
    wo_f = np.asarray(Wo, dtype=np.float32)
    wos = [np.ascontiguousarray(wo_f[g * QD:(g + 1) * QD, :]).astype(BF16)
           for g in range(2)]

    in_maps = []
    for c in range(NCORES):
        b, g = c // 2, c % 2
        xT_b = np.ascontiguousarray(x[b].T).astype(BF16)   # [H, S]
        in_maps.append(
            {
                "xT": xT_b,
                "wq": wqs[g],
                "wk": wks[g],
                "wv": wvs[g],
                "wo": wos[g],
                "cosk": cosT.astype(BF16),
                "sink": sinT_eff.astype(BF16),
                "maskT": maskT,
            }
        )
    return in_maps, None


def _assemble(results):
    full = np.empty((B, S, H), dtype=np.float32)
    for b in range(B):
        np.add(
            results[2 * b]["out"].astype(np.float32),
            results[2 * b + 1]["out"].astype(np.float32),
            out=full[b],
        )
    return full


LAST_RESULTS = None


def kernel(x, cos, sin, Wq, Wk, Wv, Wo, _trace=False):
    global LAST_RESULTS
    from concourse.bass_utils import run_bass_kernel_spmd

    in_maps, _ = _host_prep(x, cos, sin, Wq, Wk, Wv, Wo)
    res = run_bass_kernel_spmd(
        _program(),
        in_maps,
        core_ids=list(range(NCORES)),
        trace=_trace,
        trace_cores=list(range(NCORES)) if _trace else None,
    )
    LAST_RESULTS = res
    return _assemble(res.results)
